# revision 11
# baseline (speedup 1.0000x reference)
"""Weighted BCE loss (nn_BCELoss_with_weight) on 8 Trainium2 NeuronCores.

Reference computes:
    log_p   = max(log(pred), -100)
    log_1mp = max(log1p(-pred), -100)
    bce     = -(true*log_p + (1-true)*log_1mp)    # [B,C,D,H,W] = [2,16,64,128,128]
    per_class = mean(bce, axes=(0,2,3,4))         # [C]
    out = sum(weight*per_class) / sum(weight)     # scalar

Sharding: D=64 split into 8 slices of 8 (data parallel). Per core the shard
[2,16,8,128,128] is laid out as [(C,Dl)=128 partitions, B*H*W=32768 free]:
partition p holds class c=p//8 only, so the class weight is a per-partition
scalar. Both inputs are cast on host during sharding (pred -> bf16 clamped
to <= 1-2^-8 so Ln(1-p) stays finite, true -> bf16), halving HBM traffic to
16 MiB/core; the bf16-pred curvature bias in ln(1-p) is ~1.6e-3 relative
(validated against the f32 reference; tolerance is 2e-2).

Per core on device, for each column chunk (u=ln(p), v=ln(1-p), t=true):
    ACT : u = Ln(p) [bf16 out];  v = Ln(-p+1) [bf16, accum_out -> sum(v)]
    DVE : u <- u - v (bf16 TT);  u <- t * u  (bf16 TT)
    PE  : psum[1,512] += wf[128,1].T @ (t*(u-v))[:, 512-chunk]  (f32 acc)
    out_m[1,1]  = sum(psum)          -- already class-weighted
    out_v[128,1] = per-partition sum(v)
Host: result = -(sum_cores out_m + sum_p wf[p]*out_v[p]) / (M*sum(wf)),
M = B*D*H*W, f64, with the bf16-rounded weights used consistently in
numerator and denominator (exact weighted mean w.r.t. the bf16 weights).

Engine budget per core (measured rates): ACT 2 Ln passes x 32768 cols @
~0.96ns/col ~ 67us (bottleneck); DVE 2 TT bf16 @ 0.52ns/col ~ 34us; PE 64
matmuls ~ 45us; DMA 16 MiB ~ 50us -- all hidden under ACT.
"""

import numpy as np

N_CORES = 8
B, C, D, H, W = 2, 16, 64, 128, 128
HW = H * W
P = 128               # (C=16) x (D_local=8) partitions
D_LOCAL = D // N_CORES
FREE = B * HW         # 32768 free elems per partition
MM_N = 512            # one PSUM bank of f32

# largest bf16 < 1.0; pred is clamped here so Ln(1-p) >= Ln(2^-8)
BF16_MAX_LT1 = 1.0 - 2.0 ** -8


def _chunk_plan(free, head, tail, mid_chunk):
    """Column chunk sizes: tapered at stream start and end, mid filled."""
    head, tail = list(head), list(tail)
    mid_total = free - sum(head) - sum(tail)
    assert mid_total >= 0, (free, head, tail)
    n_mid = max(1, -(-mid_total // mid_chunk))
    base = mid_total // n_mid
    rem = mid_total - base * n_mid
    mids = [base + (1 if i < rem else 0) for i in range(n_mid)]
    plan = head + [m for m in mids if m] + tail
    assert sum(plan) == free
    return plan


def build_bass_kernel(free=FREE,
                      head=(1024, 2048, 4096), tail=(1024,),
                      mid_chunk=8192, p_bufs=3, t_bufs=2, uv_bufs=3,
                      split_rings=False):
    """Build the per-core Bass/Tile kernel.

    Inputs  : pred, true [128, free] bf16 (host-cast)
              wf [128, 1] bf16 (per-partition class weight)
    Outputs : out [128, 2] f32: col 0 = per-partition sum_e v[p, e];
              out[0, 1] = sum_p wf[p] * sum_e (t*(u-v))[p, e]
    """
    import concourse.bacc as bacc
    import concourse.mybir as mybir
    import concourse.tile as tile

    f32 = mybir.dt.float32
    bf16 = mybir.dt.bfloat16
    AF = mybir.ActivationFunctionType

    chunks = _chunk_plan(free, head, tail, mid_chunk)
    ncols = len(chunks)
    total_mm = sum(seg // MM_N for seg in chunks)

    nc = bacc.Bacc("TRN2", target_bir_lowering=False, debug=False,
                   num_devices=N_CORES)
    pred_d = nc.dram_tensor("pred", [P, free], bf16, kind="ExternalInput")
    true_d = nc.dram_tensor("true", [P, free], bf16, kind="ExternalInput")
    wf_d = nc.dram_tensor("wf", [P, 1], bf16, kind="ExternalInput")
    out_d = nc.dram_tensor("out", [P, 2], f32, kind="ExternalOutput")

    with tile.TileContext(nc) as tc:
        with (
            tc.tile_pool(name="pin", bufs=p_bufs) as pin,
            tc.tile_pool(name="tin", bufs=t_bufs) as tin,
            tc.tile_pool(name="uv", bufs=uv_bufs) as uvp,
            tc.tile_pool(name="small", bufs=1) as small,
            tc.tile_pool(name="psum", bufs=1, space="PSUM") as psump,
        ):
            # first input chunks go on the wire before anything else so the
            # ACT stream can start as early as possible
            seg0 = chunks[0]
            p0 = pin.tile([P, seg0], bf16, tag="p")
            t0 = tin.tile([P, seg0], bf16, tag="t")
            nc.sync.dma_start(p0[:], pred_d[:, 0:seg0])
            nc.sync.dma_start(t0[:], true_d[:, 0:seg0])
            wf_t = small.tile([P, 1], bf16, tag="wf")
            nc.sync.dma_start(wf_t[:], wf_d[:])
            vacc = small.tile([P, ncols], f32, tag="vacc")
            acc_m = psump.tile([1, MM_N], f32, tag="acc_m")
            # warm up the Ln table set so the first real ACTIVATE doesn't
            # pay the ACT_TABLE_LOAD after its data lands
            w0 = small.tile([P, 1], f32, tag="warm0")
            nc.vector.memset(w0[:], 1.0)
            wt = small.tile([P, 1], bf16, tag="warm")
            nc.scalar.activation(wt[:], w0[:], AF.Ln, bias=0.0, scale=1.0)

            off = 0
            mm_i = 0
            for ci, seg in enumerate(chunks):
                sl = slice(off, off + seg)
                if ci == 0:
                    p_t, t_t = p0, t0
                else:
                    p_t = pin.tile([P, seg], bf16, tag="p")
                    t_t = tin.tile([P, seg], bf16, tag="t")
                    p_eng = nc.scalar if (split_rings and ci % 2) else nc.sync
                    p_eng.dma_start(p_t[:], pred_d[:, sl])
                    t_eng = (nc.scalar if (split_rings and ci % 2 == 0)
                             else nc.sync)
                    t_eng.dma_start(t_t[:], true_d[:, sl])
                u = uvp.tile([P, seg], bf16, tag="u")
                v = uvp.tile([P, seg], bf16, tag="v")
                # u = ln(p); v = ln(1 - p), vacc col <- sum(v)
                nc.scalar.activation(u[:], p_t[:], AF.Ln, bias=0.0, scale=1.0)
                nc.scalar.activation(v[:], p_t[:], AF.Ln, bias=1.0, scale=-1.0,
                                     accum_out=vacc[:, ci:ci + 1])
                # u <- d = u - v ; u <- m = t * d   (bf16 TT)
                nc.vector.tensor_sub(u[:], u[:], v[:])
                nc.vector.tensor_mul(u[:], t_t[:], u[:])
                # acc_m[1, 512] += wf.T @ m[:, 512-chunk]
                for q in range(seg // MM_N):
                    nc.tensor.matmul(
                        acc_m[:],
                        wf_t[:],
                        u[:, q * MM_N:(q + 1) * MM_N],
                        start=(mm_i == 0),
                        stop=(mm_i == total_mm - 1),
                    )
                    mm_i += 1
                off += seg
            assert off == free and mm_i == total_mm

            out_t = small.tile([P, 2], f32, tag="out")
            nc.vector.memset(out_t[:, 1:2], 0.0)
            nc.vector.reduce_sum(out_t[:, 0:1], vacc[:],
                                 axis=mybir.AxisListType.X)
            accm_sb = small.tile([1, MM_N], f32, tag="accm_sb")
            nc.vector.tensor_copy(accm_sb[:], acc_m[:])
            nc.vector.reduce_sum(out_t[0:1, 1:2], accm_sb[:],
                                 axis=mybir.AxisListType.X)
            nc.sync.dma_start(out_d[:], out_t[:])

    nc.compile()
    return nc


_NC_CACHE = {}


def _get_nc():
    if "nc" not in _NC_CACHE:
        import json
        import os

        opts = json.loads(os.environ.get("KERNEL_OPTS", "{}"))
        for k in ("head", "tail"):
            if k in opts:
                opts[k] = tuple(opts[k])
        _NC_CACHE["nc"] = build_bass_kernel(**opts)
    return _NC_CACHE["nc"]


def _bf16_round(x):
    """Round f32 array to bf16 values (kept in f32 representation)."""
    xi = np.asarray(x, dtype=np.float32).view(np.uint32)
    rounded = ((xi + 0x7FFF + ((xi >> 16) & 1)) & 0xFFFF0000).astype(np.uint32)
    return rounded.view(np.float32)


def shard_inputs(pred, true, weight):
    """Full [B,C,D,H,W] f32 -> per-core in_maps of [128, 32768] bf16."""
    import ml_dtypes

    bf16 = ml_dtypes.bfloat16
    wtile = np.repeat(np.asarray(weight, np.float32), D_LOCAL).reshape(P, 1)
    wf = wtile.astype(bf16)
    pb = np.minimum(np.asarray(pred, np.float32).astype(bf16),
                    bf16(BF16_MAX_LT1))
    tb = np.asarray(true, np.float32).astype(bf16)

    def lay(x):
        # [B,C,cores,Dl,HW] -> [cores, C, Dl, B, HW] -> [cores, 128, B*HW]
        x = x.reshape(B, C, N_CORES, D_LOCAL, HW)
        x = np.ascontiguousarray(x.transpose(2, 1, 3, 0, 4))
        return x.reshape(N_CORES, P, FREE)

    ps, ts = lay(pb), lay(tb)
    return [{"pred": ps[i], "true": ts[i], "wf": wf}
            for i in range(N_CORES)]


def combine(out_ms, out_vs, weight):
    """out_ms [n_cores] scalars, out_vs [n_cores, 128]; weight [16] f32."""
    wt = _bf16_round(np.repeat(np.asarray(weight, np.float32), D_LOCAL))
    wt64 = wt.astype(np.float64)
    m = float(B * D * H * W)
    w_sum = wt64[::D_LOCAL].sum()          # sum of the 16 bf16 class weights
    total_v = (np.asarray(out_vs, np.float64).sum(axis=0) * wt64).sum()
    total_m = float(np.asarray(out_ms, np.float64).sum())
    return np.float32(-(total_m + total_v) / (m * w_sum))


def kernel(pred, true, weight, _trace=False):
    from concourse.bass_utils import run_bass_kernel_spmd

    nc = _get_nc()
    in_maps = shard_inputs(np.asarray(pred), np.asarray(true), weight)
    res = run_bass_kernel_spmd(nc, in_maps, core_ids=list(range(N_CORES)),
                               trace=_trace)
    out_ms = [r["out"][0, 1] for r in res.results]
    out_vs = [r["out"][:, 0] for r in res.results]
    out = combine(out_ms, out_vs, weight)
    if _trace:
        return out, res
    return out


# revision 14
# speedup vs baseline: 1.1398x; 1.1398x over previous
"""Weighted BCE loss (nn_BCELoss_with_weight) on 8 Trainium2 NeuronCores.

Reference computes:
    log_p   = max(log(pred), -100)
    log_1mp = max(log1p(-pred), -100)
    bce     = -(true*log_p + (1-true)*log_1mp)    # [B,C,D,H,W] = [2,16,64,128,128]
    per_class = mean(bce, axes=(0,2,3,4))         # [C]
    out = sum(weight*per_class) / sum(weight)     # scalar

Sharding: D=64 split into 8 slices of 8 (data parallel). Per core the shard
[2,16,8,128,128] is laid out as [(C,Dl)=128 partitions, B*H*W=32768 free]:
partition p holds class c=p//8 only, so the class weight is a per-partition
scalar. Both inputs are cast on host during sharding (pred -> bf16 clamped
to <= 1-2^-8 so Ln(1-p) stays finite, true -> bf16), halving HBM traffic to
16 MiB/core; the bf16-pred curvature bias in ln(1-p) is ~1.6e-3 relative
(validated against the f32 reference; tolerance is 2e-2).

Per core on device, for each column chunk (u=ln(p), v=ln(1-p), t=true):
    ACT : u = Ln(p) [bf16 out];  v = Ln(-p+1) [bf16, accum_out -> sum(v)]
    DVE : u <- u - v (bf16 TT);  u <- t * u  (bf16 TT)
    PE  : psum[1,512] += wf[128,1].T @ (t*(u-v))[:, 512-chunk]  (f32 acc)
    out_m[1,1]  = sum(psum)          -- already class-weighted
    out_v[128,1] = per-partition sum(v)
Host: result = -(sum_cores out_m + sum_p wf[p]*out_v[p]) / (M*sum(wf)),
M = B*D*H*W, f64, with the bf16-rounded weights used consistently in
numerator and denominator (exact weighted mean w.r.t. the bf16 weights).

Engine budget per core (measured rates): ACT 2 Ln passes x 32768 cols @
~0.96ns/col ~ 67us (bottleneck); DVE 2 TT bf16 @ 0.52ns/col ~ 34us; PE 64
matmuls ~ 45us; DMA 16 MiB ~ 50us -- all hidden under ACT.
"""

import numpy as np

N_CORES = 8
B, C, D, H, W = 2, 16, 64, 128, 128
HW = H * W
P = 128               # (C=16) x (D_local=8) partitions
D_LOCAL = D // N_CORES
FREE = B * HW         # 32768 free elems per partition
MM_N = 512            # one PSUM bank of f32

# largest bf16 < 1.0; pred is clamped here so Ln(1-p) >= Ln(2^-8)
BF16_MAX_LT1 = 1.0 - 2.0 ** -8


def _chunk_plan(free, head, tail, mid_chunk):
    """Column chunk sizes: tapered at stream start and end, mid filled."""
    head, tail = list(head), list(tail)
    mid_total = free - sum(head) - sum(tail)
    assert mid_total >= 0, (free, head, tail)
    n_mid = max(1, -(-mid_total // mid_chunk))
    base = mid_total // n_mid
    rem = mid_total - base * n_mid
    mids = [base + (1 if i < rem else 0) for i in range(n_mid)]
    plan = head + [m for m in mids if m] + tail
    assert sum(plan) == free
    return plan


def build_bass_kernel(free=FREE,
                      head=(1024, 2048, 4096), tail=(4096, 2048, 1024),
                      mid_chunk=8192, piece=2048,
                      p_bufs=3, t_bufs=2, uv_bufs=3, d_bufs=4,
                      p_ahead=2):
    """Build the per-core Bass/Tile kernel.

    Inputs  : pred, true [128, free] bf16 (host-cast)
              wf [128, 1] bf16 (per-partition class weight)
    Outputs : out [128, 2] f32: col 0 = per-partition sum_e v[p, e];
              out[0, 1] = sum_p wf[p] * sum_e (t*(u-v))[p, e]

    ACT works at chunk grain (large ACTIVATEs amortize fixed cost); DVE/PE
    work at `piece` grain via separate d-tiles so PE trails DVE by one
    piece, not one chunk. The p-chunk DMA stream runs `p_ahead` chunks
    ahead of the t stream (p gates ACT; t only gates the DVE mul).
    """
    import concourse.bacc as bacc
    import concourse.mybir as mybir
    import concourse.tile as tile

    f32 = mybir.dt.float32
    bf16 = mybir.dt.bfloat16
    AF = mybir.ActivationFunctionType

    chunks = _chunk_plan(free, head, tail, mid_chunk)
    ncols = len(chunks)
    offs = [sum(chunks[:i]) for i in range(ncols)]
    total_mm = sum(seg // MM_N for seg in chunks)

    nc = bacc.Bacc("TRN2", target_bir_lowering=False, debug=False,
                   num_devices=N_CORES)
    pred_d = nc.dram_tensor("pred", [P, free], bf16, kind="ExternalInput")
    true_d = nc.dram_tensor("true", [P, free], bf16, kind="ExternalInput")
    wf_d = nc.dram_tensor("wf", [P, 1], bf16, kind="ExternalInput")
    out_d = nc.dram_tensor("out", [P, 2], f32, kind="ExternalOutput")

    with tile.TileContext(nc) as tc:
        with (
            tc.tile_pool(name="pin", bufs=p_bufs) as pin,
            tc.tile_pool(name="tin", bufs=t_bufs) as tin,
            tc.tile_pool(name="uv", bufs=uv_bufs) as uvp,
            tc.tile_pool(name="dpc", bufs=d_bufs) as dpc,
            tc.tile_pool(name="small", bufs=1) as small,
            tc.tile_pool(name="psum", bufs=1, space="PSUM") as psump,
        ):
            # p chunks 0..p_ahead go on the wire before anything else; the
            # t stream trails so ACT never waits behind t transfers
            p_tiles, t_tiles = {}, {}

            def issue_p(ci):
                if ci < ncols and ci not in p_tiles:
                    p_tiles[ci] = pin.tile([P, chunks[ci]], bf16, tag="p",
                                           name=f"p{ci}")
                    nc.sync.dma_start(
                        p_tiles[ci][:],
                        pred_d[:, offs[ci]:offs[ci] + chunks[ci]])

            def issue_t(ci):
                if ci < ncols and ci not in t_tiles:
                    t_tiles[ci] = tin.tile([P, chunks[ci]], bf16, tag="t",
                                           name=f"t{ci}")
                    nc.sync.dma_start(
                        t_tiles[ci][:],
                        true_d[:, offs[ci]:offs[ci] + chunks[ci]])

            for ci in range(min(p_ahead + 1, ncols)):
                issue_p(ci)
            issue_t(0)
            wf_t = small.tile([P, 1], bf16, tag="wf")
            nc.sync.dma_start(wf_t[:], wf_d[:])
            vacc = small.tile([P, ncols], f32, tag="vacc")
            acc_m = psump.tile([1, MM_N], f32, tag="acc_m")
            # warm up the Ln table set so the first real ACTIVATE doesn't
            # pay the ACT_TABLE_LOAD after its data lands
            w0 = small.tile([P, 1], f32, tag="warm0")
            nc.vector.memset(w0[:], 1.0)
            wt = small.tile([P, 1], bf16, tag="warm")
            nc.scalar.activation(wt[:], w0[:], AF.Ln, bias=0.0, scale=1.0)

            mm_i = 0
            for ci, seg in enumerate(chunks):
                issue_p(ci + p_ahead)
                issue_t(ci + 1)
                p_t, t_t = p_tiles.pop(ci), t_tiles.pop(ci)
                u = uvp.tile([P, seg], bf16, tag="u")
                v = uvp.tile([P, seg], bf16, tag="v")
                # u = ln(p); v = ln(1 - p), vacc col <- sum(v)
                nc.scalar.activation(u[:], p_t[:], AF.Ln, bias=0.0, scale=1.0)
                nc.scalar.activation(v[:], p_t[:], AF.Ln, bias=1.0, scale=-1.0,
                                     accum_out=vacc[:, ci:ci + 1])
                # per piece: d = u - v; d <- t * d; psum += wf.T @ d
                for ps in range(0, seg, piece):
                    pw = min(piece, seg - ps)
                    sub_sl = slice(ps, ps + pw)
                    d = dpc.tile([P, pw], bf16, tag="d")
                    nc.vector.tensor_sub(d[:], u[:, sub_sl], v[:, sub_sl])
                    nc.vector.tensor_mul(d[:], t_t[:, sub_sl], d[:])
                    for q in range(pw // MM_N):
                        nc.tensor.matmul(
                            acc_m[:],
                            wf_t[:],
                            d[:, q * MM_N:(q + 1) * MM_N],
                            start=(mm_i == 0),
                            stop=(mm_i == total_mm - 1),
                        )
                        mm_i += 1
            assert mm_i == total_mm

            out_t = small.tile([P, 2], f32, tag="out")
            nc.vector.memset(out_t[:, 1:2], 0.0)
            nc.vector.reduce_sum(out_t[:, 0:1], vacc[:],
                                 axis=mybir.AxisListType.X)
            # final psum[1,512] -> scalar
            accm_sb = small.tile([1, MM_N], f32, tag="accm_sb")
            nc.vector.tensor_copy(accm_sb[:], acc_m[:])
            nc.vector.reduce_sum(out_t[0:1, 1:2], accm_sb[:],
                                 axis=mybir.AxisListType.X)
            nc.sync.dma_start(out_d[:], out_t[:])

    nc.compile()
    return nc


_NC_CACHE = {}


def _get_nc():
    if "nc" not in _NC_CACHE:
        import json
        import os

        opts = json.loads(os.environ.get("KERNEL_OPTS", "{}"))
        for k in ("head", "tail"):
            if k in opts:
                opts[k] = tuple(opts[k])
        _NC_CACHE["nc"] = build_bass_kernel(**opts)
    return _NC_CACHE["nc"]


def _bf16_round(x):
    """Round f32 array to bf16 values (kept in f32 representation)."""
    xi = np.asarray(x, dtype=np.float32).view(np.uint32)
    rounded = ((xi + 0x7FFF + ((xi >> 16) & 1)) & 0xFFFF0000).astype(np.uint32)
    return rounded.view(np.float32)


def shard_inputs(pred, true, weight):
    """Full [B,C,D,H,W] f32 -> per-core in_maps of [128, 32768] bf16."""
    import ml_dtypes

    bf16 = ml_dtypes.bfloat16
    wtile = np.repeat(np.asarray(weight, np.float32), D_LOCAL).reshape(P, 1)
    wf = wtile.astype(bf16)
    pb = np.minimum(np.asarray(pred, np.float32).astype(bf16),
                    bf16(BF16_MAX_LT1))
    tb = np.asarray(true, np.float32).astype(bf16)

    def lay(x):
        # [B,C,cores,Dl,HW] -> [cores, C, Dl, B, HW] -> [cores, 128, B*HW]
        x = x.reshape(B, C, N_CORES, D_LOCAL, HW)
        x = np.ascontiguousarray(x.transpose(2, 1, 3, 0, 4))
        return x.reshape(N_CORES, P, FREE)

    ps, ts = lay(pb), lay(tb)
    return [{"pred": ps[i], "true": ts[i], "wf": wf}
            for i in range(N_CORES)]


def combine(out_ms, out_vs, weight):
    """out_ms [n_cores] scalars, out_vs [n_cores, 128]; weight [16] f32."""
    wt = _bf16_round(np.repeat(np.asarray(weight, np.float32), D_LOCAL))
    wt64 = wt.astype(np.float64)
    m = float(B * D * H * W)
    w_sum = wt64[::D_LOCAL].sum()          # sum of the 16 bf16 class weights
    total_v = (np.asarray(out_vs, np.float64).sum(axis=0) * wt64).sum()
    total_m = float(np.asarray(out_ms, np.float64).sum())
    return np.float32(-(total_m + total_v) / (m * w_sum))


def kernel(pred, true, weight, _trace=False):
    from concourse.bass_utils import run_bass_kernel_spmd

    nc = _get_nc()
    in_maps = shard_inputs(np.asarray(pred), np.asarray(true), weight)
    res = run_bass_kernel_spmd(nc, in_maps, core_ids=list(range(N_CORES)),
                               trace=_trace)
    out_ms = [r["out"][0, 1] for r in res.results]
    out_vs = [r["out"][:, 0] for r in res.results]
    out = combine(out_ms, out_vs, weight)
    if _trace:
        return out, res
    return out
